# revision 6
# baseline (speedup 1.0000x reference)
"""Trainium2 Bass kernel for nn_Encoder_P: unwrap-diff-square front-end + 4 dilated
convs with dense concatenation, fused end-to-end on-chip.

Strategy (pure data parallel, 1 batch sample per NeuronCore, 8 cores):
  - The unwrap/diff/pad chain collapses: cumsum cancels in the diff, so
    sq[h] = wrap(p[h] - p[h-1])^2 (row 0 = 0), wrap(v) = v - 2*pi*k with
    k = (v>=pi) + (v>=3pi) - (v<=-pi) - (v<=-3pi).
  - Duplicate concat channels are folded into effective conv weights
    (conv3: 8->7 input planes, conv4: 20->15).
  - Each conv runs on TensorE as banded matmuls over the H (partition) axis:
    lhsT is a banded [128,128] H-shift matrix built on-device (DVE) from 5
    shared shifted-identity masters scaled by runtime weight scalars; rhs is
    the input plane tile [128 H, 516 Wpad]; PSUM accumulates over (ci, kw).
  - Planes are stored as 5 overlapping H-tiles (stride 104, halo 12) of
    [128, 516] with zeroed W margins, so conv H/W reach never crosses a tile.
"""

import numpy as np

import concourse.bacc as bacc
import concourse.bass as bass
import concourse.mybir as mybir
import concourse.tile as tile
from concourse import bass_utils

F32 = mybir.dt.float32
MM_DT = mybir.dt.float32r  # full-rate fp32 matmul path (1 cyc/row at N>=256)
DEFAULT_MM = "f32r"  # flip to "bf16" only with HW-validated accuracy+speed

H = 512
W = 512
S = 107          # tile stride in rows (chosen so 512-(S*4-HALO) == 96, a legal
                 # compute-op partition start for the bottom edge-zero memset)
HALO = 12        # halo rows above/below each tile
NT = 5           # number of H tiles
WPAD = 516       # 2 zero cols + 512 + 2 zero cols
P = 128
PI = float(np.pi)

# conv specs: (dil, pad_top, pad_left, KH, KW)
CONV_GEOM = [
    (1, 1, 1, 4, 4),   # conv1: 4x4 dil1, 'same' pad (1,2)
    (2, 2, 2, 3, 3),   # conv2: 3x3 dil2, pad (2,2)
    (3, 1, 1, 2, 2),   # conv3: 2x2 dil3, pad (1,2)
    (4, 0, 0, 1, 1),   # conv4: 1x1
]

PLANE_NAMES = (
    ["sq", "c1_0", "c1_1"]
    + [f"c2_{i}" for i in range(4)]
    + [f"c3_{i}" for i in range(8)]
)
CONV_INPUTS = [
    ["sq"],
    ["c1_0", "c1_1", "sq"],
    [f"c2_{i}" for i in range(4)] + ["c1_0", "c1_1", "sq"],
    [f"c3_{i}" for i in range(8)] + [f"c2_{i}" for i in range(4)]
    + ["c1_0", "c1_1", "sq"],
]
CONV_OUT = [2, 4, 8, 16]
DELTAS = [-2, -1, 0, 1, 2]  # identity master shifts

# output channel -> source plane ("c4_o" channels handled separately)
CH_MAP = (
    [f"c4_{i}" for i in range(16)]
    + [f"c3_{i}" for i in range(8)]
    + [f"c2_{i}" for i in range(4)]
    + ["c1_0", "c1_1", "sq", "sq", "c1_0", "c1_1", "sq", "sq"]
    + [f"c2_{i}" for i in range(4)]
    + ["c1_0", "c1_1", "sq", "sq"]
    + ["c1_0", "c1_1", "sq", "sq"]
)

NSCAL = sum(
    CONV_OUT[c] * len(CONV_INPUTS[c]) * CONV_GEOM[c][3] * CONV_GEOM[c][4]
    for c in range(4)
)  # 604

# device writes 31 unique channels; host expands to 48 via this index map
_UNIQ_OF = (
    {f"c3_{i}": 16 + i for i in range(8)}
    | {f"c2_{i}": 24 + i for i in range(4)}
    | {"c1_0": 28, "c1_1": 29, "sq": 30}
)
DUP48 = list(range(16)) + [_UNIQ_OF[CH_MAP[ch]] for ch in range(16, 48)]


def _fold_weights(w1, w2, w3, w4):
    w3f = np.zeros((8, 7, 2, 2), np.float32)
    w3f[:, :6] = w3[:, :6]
    w3f[:, 6] = w3[:, 6] + w3[:, 7]
    w4f = np.zeros((16, 15, 1, 1), np.float32)
    w4f[:, :12] = w4[:, :12]
    w4f[:, 12] = w4[:, 12] + w4[:, 16]
    w4f[:, 13] = w4[:, 13] + w4[:, 17]
    w4f[:, 14] = w4[:, 14] + w4[:, 15] + w4[:, 18] + w4[:, 19]
    return [w1.astype(np.float32), w2.astype(np.float32), w3f, w4f]


def _host_tables(inputs):
    """wtab [128, NSCAL], ident [5*128, 128], bias [128, 30] host arrays."""
    wf = _fold_weights(inputs["w1"], inputs["w2"], inputs["w3"], inputs["w4"])
    scal = []
    for c in range(4):
        dil, pad_top, _, KH, KW = CONV_GEOM[c]
        for o in range(CONV_OUT[c]):
            for ci in range(len(CONV_INPUTS[c])):
                for kw in range(KW):
                    for kh in range(KH):
                        scal.append(wf[c][o, ci, kh, kw])
    assert len(scal) == NSCAL
    wtab = np.tile(np.asarray(scal, np.float32)[None, :], (P, 1))
    ident = np.concatenate(
        [np.eye(P, dtype=np.float32, k=-d) for d in DELTAS], axis=0
    )
    bias = np.concatenate(
        [inputs["b1"], inputs["b2"], inputs["b3"], inputs["b4"]]
    ).astype(np.float32)
    bias = np.tile(bias[None, :], (P, 1))
    return wtab, ident, bias


def build_nc(loop_k=1, out_mode='full', skip_bands=False, mm='f32r'):
    nc = bacc.Bacc("TRN2", target_bir_lowering=False, debug=False)
    mm_dt = mybir.dt.bfloat16 if mm == 'bf16' else MM_DT

    def msafe(ap):
        # memset target: walrus rejects float32r memsets; bitcast those to f32
        return ap.bitcast(F32) if mm != 'bf16' else ap

    p_dram = nc.dram_tensor("p", [H, W], F32, kind="ExternalInput")
    ident_dram = nc.dram_tensor("ident", [5 * P, P], F32, kind="ExternalInput")
    wtab_dram = nc.dram_tensor("wtab", [P, NSCAL], F32, kind="ExternalInput")
    bias_dram = nc.dram_tensor("bias", [P, 30], F32, kind="ExternalInput")
    out_dram = nc.dram_tensor("out", [31, H, W], F32, kind="ExternalOutput")

    planes = {
        nm: nc.alloc_sbuf_tensor(f"pl_{nm}", [P, NT * WPAD], mm_dt)
        for nm in PLANE_NAMES
    }
    ident_sb = nc.alloc_sbuf_tensor("ident_sb", [P, 5 * P], F32)
    wtab_sb = nc.alloc_sbuf_tensor("wtab_sb", [P, NSCAL], F32)
    bias_sb = nc.alloc_sbuf_tensor("bias_sb", [P, 30], F32)

    def pslice(nm, t, c0, c1):
        return planes[nm][:, t * WPAD + c0 : t * WPAD + c1]

    with tile.TileContext(nc) as tc:
        with (
            tc.tile_pool(name="io", bufs=3) as io_pool,
            tc.tile_pool(name="front", bufs=2) as fr_pool,
            tc.tile_pool(name="bands", bufs=12) as band_pool,
            tc.tile_pool(name="psum", bufs=8, space="PSUM") as psum_pool,
            tc.tile_pool(name="c4st", bufs=3) as c4_pool,
        ):
            for _it in range(loop_k):
                # ---- parameter loads ----
                for j in range(5):
                    nc.sync.dma_start(
                        out=ident_sb[:, j * P : (j + 1) * P],
                        in_=ident_dram[j * P : (j + 1) * P, :],
                    )
                nc.sync.dma_start(out=wtab_sb[:], in_=wtab_dram[:])
                nc.sync.dma_start(out=bias_sb[:], in_=bias_dram[:])

                # ---- zero W margins of all planes (written once) ----
                for nm in PLANE_NAMES:
                    for t in range(NT):
                        nc.gpsimd.memset(msafe(pslice(nm, t, 0, 2)), 0.0)
                        nc.gpsimd.memset(msafe(pslice(nm, t, 514, 516)), 0.0)

                # ---- front-end: sq ----
                # A/B garbage regions are pre-zeroed so the out-of-image rows
                # compute v=0 -> sq=0, which is exactly the reference's zero pad.
                for t in range(NT):
                    p_lo = HALO if t == 0 else 0
                    p_hi = H - (S * (NT - 1) - HALO) if t == NT - 1 else P  # 96 at t=4
                    n = p_hi - p_lo
                    r_lo = S * t - HALO + p_lo
                    A = io_pool.tile([P, W], F32, tag="A")
                    B = io_pool.tile([P, W], F32, tag="B")
                    if t == 0:
                        nc.gpsimd.memset(A[0:32, :], 0.0)
                        nc.gpsimd.memset(B[0:32, :], 0.0)
                    if t == NT - 1:
                        nc.gpsimd.memset(A[96:P, :], 0.0)
                        nc.gpsimd.memset(B[96:P, :], 0.0)
                    nc.sync.dma_start(out=A[p_lo:p_hi, :], in_=p_dram[r_lo : r_lo + n, :])
                    if t == 0:
                        nc.sync.dma_start(
                            out=B[p_lo + 1 : p_hi, :], in_=p_dram[0 : n - 1, :]
                        )
                        nc.sync.dma_start(out=B[p_lo : p_lo + 1, :], in_=p_dram[0:1, :])
                    else:
                        nc.sync.dma_start(
                            out=B[p_lo:p_hi, :], in_=p_dram[r_lo - 1 : r_lo - 1 + n, :]
                        )
                    V = fr_pool.tile([P, W], F32, tag="V")
                    K1 = fr_pool.tile([P, W], F32, tag="K1")
                    K2 = fr_pool.tile([P, W], F32, tag="K2")
                    K3 = fr_pool.tile([P, W], F32, tag="K3")
                    K4 = fr_pool.tile([P, W], F32, tag="K4")
                    ao = mybir.AluOpType
                    nc.vector.tensor_tensor(V[:], A[:], B[:], ao.subtract)
                    nc.vector.tensor_scalar(K1[:], V[:], PI, None, ao.is_ge)
                    nc.vector.tensor_scalar(K2[:], V[:], 3 * PI, None, ao.is_ge)
                    nc.vector.tensor_scalar(K3[:], V[:], -PI, None, ao.is_le)
                    nc.vector.tensor_scalar(K4[:], V[:], -3 * PI, None, ao.is_le)
                    nc.vector.tensor_tensor(K1[:], K1[:], K2[:], ao.add)
                    nc.vector.tensor_tensor(K3[:], K3[:], K4[:], ao.add)
                    nc.vector.tensor_tensor(K1[:], K1[:], K3[:], ao.subtract)
                    nc.vector.scalar_tensor_tensor(
                        V[:], K1[:], -2 * PI, V[:], ao.mult, ao.add
                    )
                    sq_dst = planes["sq"][:, t * WPAD + 2 : t * WPAD + 514]
                    nc.vector.tensor_tensor(sq_dst, V[:], V[:], ao.mult)

                # ---- convs ----
                jcol = 0
                bias_col = 0
                p_hi_last = H - (S * (NT - 1) - HALO)  # 108
                for c in range(4):
                    dil, pad_top, pad_left, KH, KW = CONV_GEOM[c]
                    in_names = CONV_INPUTS[c]
                    O = CONV_OUT[c]
                    deltas = [kh * dil - pad_top for kh in range(KH)]
                    for o in range(O):
                        psums = [
                            psum_pool.tile([P, W], F32, tag="ps", name=f"ps_{c}_{o}_{t}")
                            for t in range(NT)
                        ]
                        for ci, nm in enumerate(in_names):
                            for kw in range(KW):
                                band = band_pool.tile([P, P], mm_dt, tag="band")
                                if skip_bands:
                                    deltas_eff = []
                                    jcol += len(deltas)
                                else:
                                    deltas_eff = deltas
                                for i, d in enumerate(deltas_eff):
                                    w_ap = wtab_sb[:, jcol : jcol + 1]
                                    jcol += 1
                                    src = ident_sb[
                                        :, (d + 2) * P : (d + 3) * P
                                    ]
                                    ao = mybir.AluOpType
                                    if i == 0:
                                        nc.vector.tensor_scalar(
                                            band[:], src, w_ap, None, ao.mult
                                        )
                                    else:
                                        nc.vector.scalar_tensor_tensor(
                                            band[:], src, w_ap, band[:], ao.mult, ao.add
                                        )
                                coff = 2 + kw * dil - pad_left
                                first = ci == 0 and kw == 0
                                last = ci == len(in_names) - 1 and kw == KW - 1
                                for t in range(NT):
                                    rhs = planes[nm][
                                        :, t * WPAD + coff : t * WPAD + coff + W
                                    ]
                                    nc.tensor.matmul(
                                        psums[t],
                                        (
                                            ident_sb[:, 2 * P : 3 * P].bitcast(mm_dt)
                                            if mm != "bf16"
                                            else ident_sb[:, 2 * P : 3 * P]
                                        )
                                        if skip_bands
                                        else band[:],
                                        rhs,
                                        start=first,
                                        stop=last,
                                    )
                        bias_ap = bias_sb[:, bias_col + o : bias_col + o + 1]
                        if c < 3:
                            out_nm = (
                                ["c1_0", "c1_1"][o]
                                if c == 0
                                else (f"c2_{o}" if c == 1 else f"c3_{o}")
                            )
                            for t in range(NT):
                                nc.scalar.add(
                                    pslice(out_nm, t, 2, 514), psums[t][:], bias_ap
                                )
                        else:
                            for t in range(NT):
                                st = c4_pool.tile([P, W], F32, tag="c4")
                                nc.scalar.add(st[:], psums[t][:], bias_ap)
                                rows = S if t < NT - 1 else H - S * (NT - 1)
                                nc.sync.dma_start(
                                    out=out_dram[o, S * t : S * t + rows, :],
                                    in_=st[HALO : HALO + rows, :],
                                )
                    # edge-zero the new planes (reference 'same' zero padding)
                    if c < 3:
                        outs = (
                            ["c1_0", "c1_1"]
                            if c == 0
                            else (
                                [f"c2_{i}" for i in range(4)]
                                if c == 1
                                else [f"c3_{i}" for i in range(8)]
                            )
                        )
                        for nm in outs:
                            nc.gpsimd.memset(msafe(planes[nm][0:HALO, 0:WPAD]), 0.0)
                            nc.gpsimd.memset(
                                msafe(
                                    planes[nm][
                                        p_hi_last:P, (NT - 1) * WPAD : NT * WPAD
                                    ]
                                ),
                                0.0,
                            )
                    bias_col += O

                # ---- remaining unique output channels from stored planes ----
                # (channels 31..47 are duplicates of 16..30; host replicates)
                for ch in range(16, 31 if out_mode == 'full' else 16):
                    nm = CH_MAP[ch]
                    for t in range(NT):
                        rows = S if t < NT - 1 else H - S * (NT - 1)
                        src_ap = planes[nm][
                            HALO : HALO + rows, t * WPAD + 2 : t * WPAD + 514
                        ]
                        if mm == 'bf16':
                            nc.gpsimd.dma_start(
                                out=out_dram[ch, S * t : S * t + rows, :],
                                in_=src_ap,
                            )
                        else:
                            nc.sync.dma_start(
                                out=out_dram[ch, S * t : S * t + rows, :],
                                in_=src_ap.bitcast(F32),
                            )

    nc.compile()
    return nc


_NC_CACHE = None


def _get_nc():
    global _NC_CACHE
    if _NC_CACHE is None:
        _NC_CACHE = build_nc(mm=DEFAULT_MM)
    return _NC_CACHE


OUT_NAMES = ["out"]


def _core_in_maps(inputs):
    wtab, ident, bias = _host_tables(inputs)
    feat = inputs["feature_in"].astype(np.float32)  # [8,1,512,512]
    return [
        {"p": feat[b, 0], "ident": ident, "wtab": wtab, "bias": bias}
        for b in range(feat.shape[0])
    ]


def _assemble48(outs):
    """Device outputs (one core) -> [48, H, W] float32."""
    return np.ascontiguousarray(outs["out"][DUP48], dtype=np.float32)


def _run(inputs, trace=False):
    inputs = {k: np.asarray(v) for k, v in inputs.items()}
    nc = _get_nc()
    in_maps = _core_in_maps(inputs)
    n_cores = len(in_maps)
    res = bass_utils.run_bass_kernel_spmd(
        nc, in_maps, core_ids=list(range(n_cores)), trace=trace
    )
    out = np.stack(
        [_assemble48(res.results[b]) for b in range(n_cores)], axis=0
    )
    return out, res


def kernel(**inputs):
    return _run(inputs, trace=False)[0]



# revision 7
# speedup vs baseline: 1.2629x; 1.2629x over previous
"""Trainium2 Bass kernel v3 for nn_Encoder_P — grouped channel-major convs.

Structure (per core = one batch sample):
  - Front-end: sq = wrap(diff_H(p))^2 computed H-major into sq_full (f32)
    and sq_bf (bf16 out copy).
  - Convs run as dense TensorE matmuls: lhsT = host-built [K, M] weight
    tables (bias folded via an all-ones rhs row; image-edge handling folded
    into per-edge table variants), rhs = channel-interleaved "slab" of
    H-row windows, psum M packs (channel-block x row) so evictions are
    single partition-aligned copies.
  - Slabs: slab1 (conv1 in: sq windows), slab2 (conv2 in: c1_0,c1_1,sq),
    slab3 (conv3 in: c2 x4, c1 x2, sqm), slab4 (conv4 in: c3 x8, c2 x4,
    c1 x2, sq). Channel gaps are filled by sbuf->sbuf DMA (arbitrary
    partition patterns are DMA-legal; engines require quadrant alignment).
  - Image processed in 4 H-sections of 128 rows; slabs sized per-section.
  - Outputs: 31 unique channels only (c4 16 + c3 8 + c2 4 + c1 2 + sq),
    c3/c4/sq in bf16, in device-native layouts; host detangles + expands
    to the 48-channel concat.
"""

import numpy as np

import concourse.bacc as bacc
import concourse.bass as bass
import concourse.mybir as mybir
import concourse.tile as tile
from concourse import bass_utils

F32 = mybir.dt.float32
BF16 = mybir.dt.bfloat16
F32R = mybir.dt.float32r
DEFAULT_MM = "f32r"

H = 512
W = 512
P = 128
WPAD = 516
NSEC = 4          # H sections
CPS = 4           # 32-row chunks per section (conv1/slab1/slab2 granularity)
NCH = 16          # 32-row chunks per image
PI = float(np.pi)

# slab row layouts
K1 = 44           # slab1: sq rows [32k-5,+43) at 0..42, ones at 43
ONES1 = 43
K2 = 121          # slab2: c1_0 [0:40) c1_1 [40:80) sq [80:120) ones 120
ONES2 = 120
K3A = 77          # slab3a: c2_j [19j:+19) j<4, ones 76
ONES3 = 76
K3B = 57          # slab3b: c1_0 [0:19), c1_1 [19:38), sqm [38:57)
K4 = 121          # slab4: c3_j [8j:+8) j<8, c2_j [64+8j:+8), c1_0 [96:104)
ONES4 = 120       #        c1_1 [104:112), sq [112:120), ones 120

M1 = 80           # conv1 psum: o*40 + r2, windows [32k-4,+40)
M2 = 76           # conv2 psum: o*19 + r3, windows [16q-1,+19)
M3 = 128          # conv3 psum: w*64 + j*8 + r4, two 8-row windows per pair
M4 = 128          # conv4 psum: o*8 + g, windows [8m,+8)


def _fold_weights(w1, w2, w3, w4):
    w3f = np.zeros((8, 7, 2, 2), np.float32)
    w3f[:, :6] = w3[:, :6]
    w3f[:, 6] = w3[:, 6] + w3[:, 7]
    w4f = np.zeros((16, 15, 1, 1), np.float32)
    w4f[:, :12] = w4[:, :12]
    w4f[:, 12] = w4[:, 12] + w4[:, 16]
    w4f[:, 13] = w4[:, 13] + w4[:, 17]
    w4f[:, 14] = w4[:, 14] + w4[:, 15] + w4[:, 18] + w4[:, 19]
    return [np.asarray(w1, np.float32), np.asarray(w2, np.float32), w3f, w4f]


# ---- lhsT table geometry (shared by host builder and device emitter) ----

def _table_specs():
    """[(key, K, M)] in storage order; offsets derived from M."""
    specs = []
    for kw in range(4):
        for v in "ifl":
            specs.append((("c1", kw, 0, v), K1, M1))
    for kw in range(3):
        for par in range(2):
            for v in "ifl":
                specs.append((("c2", kw, par, v), K2, M2))
    for kw in range(2):
        for v in "ifl":
            specs.append((("c3a", kw, 0, v), K3A, M3))
            specs.append((("c3b", kw, 0, v), K3B, M3))
    for kw in range(3):
        for v in "ifl":
            specs.append((("c2b", kw, 0, v), K2, M3))
    specs.append((("p3b", 0, 0, "i"), K2, M3))
    specs.append((("p4g", 0, 0, "i"), K2, M3))
    specs.append((("c4", 0, 0, "i"), K4, M4))
    return specs


TBL_SPECS = _table_specs()
TBL_OFF = {}
_off = 0
for _key, _k, _m in TBL_SPECS:
    TBL_OFF[_key] = _off
    _off += _m
TBL_COLS = _off


def _build_tbl(inputs):
    """Host-built lhsT tables [128, TBL_COLS] f32."""
    wf = _fold_weights(inputs["w1"], inputs["w2"], inputs["w3"], inputs["w4"])
    bs = [np.asarray(inputs[f"b{i}"], np.float32) for i in (1, 2, 3, 4)]
    tbl = np.zeros((P, TBL_COLS), np.float32)

    def put(key, T):
        off = TBL_OFF[key]
        tbl[:, off : off + T.shape[1]] = T

    # conv1: h_out = 32k - 4 + r2 (r2 in [0,40)); h_in = h_out + kh - 1
    for kw in range(4):
        for v in "ifl":
            T = np.zeros((P, M1), np.float32)
            for o in range(2):
                for r2 in range(40):
                    ho = r2 - 4  # relative to 32k; k=0 -> ho, k=15 -> 480+ho
                    if v == "f" and ho < 0:
                        continue
                    if v == "l" and 480 + ho >= H:
                        continue
                    col = o * 40 + r2
                    for kh in range(4):
                        hi = ho + kh - 1
                        if v == "f" and hi < 0:
                            continue
                        if v == "l" and 480 + hi >= H:
                            continue
                        r1 = r2 + kh  # slab1 row: h_in - (32k - 5)
                        T[r1, col] += wf[0][o, 0, kh, kw]
                    if kw == 0:
                        T[ONES1, col] += bs[0][o]
            put(("c1", kw, 0, v), T)

    # conv2: windows q (par=q%2, slab2 chunk k=q//2):
    # h_out = 16q - 1 + r3; slab2 row base: par0 -> r2 = 3 + r3 + d,
    # par1 -> r2 = 19 + r3 + d (d = 2kh - 2)
    for kw in range(3):
        for par in range(2):
            for v in "ifl":
                T = np.zeros((P, M2), np.float32)
                base = 3 if par == 0 else 19
                for o in range(4):
                    for r3 in range(19):
                        ho = 16 * par - 1 + r3  # relative to 32k
                        if v == "f" and ho < 0:
                            continue
                        if v == "l" and 480 + ho >= H:
                            continue
                        col = o * 19 + r3
                        for ci in range(3):
                            for kh in range(3):
                                d = 2 * kh - 2
                                hi = ho + d
                                if v == "f" and hi < 0:
                                    continue
                                if v == "l" and 480 + hi >= H:
                                    continue
                                r2 = base + r3 + d
                                T[ci * 40 + r2, col] += wf[1][o, ci, kh, kw]
                        if kw == 0:
                            T[ONES2, col] += bs[1][o]
                put(("c2", kw, par, v), T)

    # conv3: pair q: h_out = 16q + w*8 + r4; d = 3kh - 1; r3 = (w*8+r4)+d+1
    # K-split: slab3a holds c2 (4ch x 19 rows + ones), slab3b c1_0/c1_1/sqm
    for kw in range(2):
        for v in "ifl":
            Ta = np.zeros((P, M3), np.float32)
            Tb = np.zeros((P, M3), np.float32)
            for j in range(8):
                for w_ in range(2):
                    for r4 in range(8):
                        ho = w_ * 8 + r4  # relative to 16q; always in-image
                        col = w_ * 64 + j * 8 + r4
                        for ci in range(7):
                            for kh in range(2):
                                d = 3 * kh - 1
                                hi = ho + d
                                if v == "f" and hi < 0:
                                    continue
                                if v == "l" and 496 + hi >= H:
                                    continue
                                r3 = ho + d + 1
                                if ci < 4:
                                    Ta[19 * ci + r3, col] += wf[2][j, ci, kh, kw]
                                else:
                                    Tb[19 * (ci - 4) + r3, col] += wf[2][
                                        j, ci, kh, kw
                                    ]
                        if kw == 0:
                            Ta[ONES3, col] += bs[2][j]
            put(("c3a", kw, 0, v), Ta)
            put(("c3b", kw, 0, v), Tb)

    # conv2 pass2 -> slab4 c2 blocks: psum rows w*32 + ci*8 + r (w<4:
    # window m = 4q + w covers h_out = 32q + 8w + r); input slab2 chunk q:
    # r2 = h_out + d - (32q - 4) = 8w + r + d + 4
    for kw in range(3):
        for v in "ifl":
            T = np.zeros((P, M3), np.float32)
            for w_ in range(4):
                for ci in range(4):
                    for r in range(8):
                        ho = 8 * w_ + r  # relative to 32q; always in-image
                        col = 32 * w_ + 8 * ci + r
                        for cin in range(3):
                            for kh in range(3):
                                d = 2 * kh - 2
                                hi = ho + d
                                if v == "f" and hi < 0:
                                    continue
                                if v == "l" and 480 + hi >= H:
                                    continue
                                T[cin * 40 + ho + d + 4, col] += wf[1][
                                    ci, cin, kh, kw
                                ]
                        if kw == 0:
                            T[ONES2, col] += bs[1][ci]
            put(("c2b", kw, 0, v), T)

    # perm p3b: gather slab2 c1_0/c1_1/sq rows -> slab3b layout for 2 windows
    # psum rows par*64 + ch*19 + r3 <- slab2 row ch*40 + r0(par) + r3
    T = np.zeros((P, M3), np.float32)
    for par in range(2):
        r0 = 3 if par == 0 else 19
        for ch in range(3):
            for r3 in range(19):
                T[ch * 40 + r0 + r3, par * 64 + ch * 19 + r3] = 1.0
    put(("p3b", 0, 0, "i"), T)

    # perm p4g: gather slab2 c1_0/c1_1/sq rows -> slab4 c1/sq blocks for 4
    # windows: psum rows w*32 + ch*8 + r <- slab2 row ch*40 + 4 + 8w + r
    T = np.zeros((P, M3), np.float32)
    for w_ in range(4):
        for ch in range(3):
            for r in range(8):
                T[ch * 40 + 4 + 8 * w_ + r, w_ * 32 + ch * 8 + r] = 1.0
    put(("p4g", 0, 0, "i"), T)

    # conv4: 1x1: h_out = 8m + g; slab4 row j*8 + g
    T = np.zeros((P, M4), np.float32)
    for o in range(16):
        for g in range(8):
            col = o * 8 + g
            for j in range(15):
                T[j * 8 + g, col] = wf[3][o, j, 0, 0]
            T[ONES4, col] = bs[3][o]
    put(("c4", 0, 0, "i"), T)
    return tbl


def _v1(k, last, first):
    return "f" if k == first else ("l" if k == last else "i")


def build_nc(loop_k=1):
    nc = bacc.Bacc("TRN2", target_bir_lowering=False, debug=False)
    ao = mybir.AluOpType

    p_dram = nc.dram_tensor("p", [H, W], F32, kind="ExternalInput")
    tbl_dram = nc.dram_tensor("tbl", [P, TBL_COLS], F32R, kind="ExternalInput")
    ones_dram = nc.dram_tensor("ones", [16 * WPAD], F32R, kind="ExternalInput")
    oc1 = nc.dram_tensor("oc1", [NSEC, 2, 32, CPS, W], F32, kind="ExternalOutput")
    oc2 = nc.dram_tensor("oc2", [NSEC, 4, 16, 2 * CPS, W], F32, kind="ExternalOutput")
    oc3 = nc.dram_tensor("oc3", [NSEC, 64, 4 * CPS, W], F32, kind="ExternalOutput")
    oc4 = nc.dram_tensor("oc4", [NSEC, P, 4 * CPS, W], BF16, kind="ExternalOutput")
    osq = nc.dram_tensor("osq", [P, 4, W], BF16, kind="ExternalOutput")

    sq_full = nc.alloc_sbuf_tensor("sq_full", [P, 4 * WPAD], F32R)
    sq_bf = nc.alloc_sbuf_tensor("sq_bf", [P, 4 * WPAD], BF16)
    slab1 = nc.alloc_sbuf_tensor("slab1", [K1, CPS * WPAD], F32R)
    slab2 = nc.alloc_sbuf_tensor("slab2", [K2, CPS * WPAD], F32R)
    slab3a = nc.alloc_sbuf_tensor("slab3a", [K3A, 2 * CPS * WPAD], F32R)
    slab3b = nc.alloc_sbuf_tensor("slab3b", [K3B, 2 * CPS * WPAD], F32R)
    slab4 = nc.alloc_sbuf_tensor("slab4", [K4, 4 * CPS * WPAD], F32R)
    obf4 = nc.alloc_sbuf_tensor("obf4", [P, 4 * CPS * WPAD], BF16)
    tbl_sb = nc.alloc_sbuf_tensor("tbl_sb", [P, TBL_COLS], F32R)

    def mm(key):
        off = TBL_OFF[key]
        kdim = {"c1": K1, "c2": K2, "c2b": K2, "p3b": K2, "p4g": K2,
                "c3a": K3A, "c3b": K3B, "c4": K4}[key[0]]
        mdim = {"c1": M1, "c2": M2, "c2b": M3, "p3b": M3, "p4g": M3,
                "c3a": M3, "c3b": M3, "c4": M4}[key[0]]
        return tbl_sb[0:kdim, off : off + mdim]

    def dcol(slab, ch, c0, c1):
        return slab[:, ch * WPAD + c0 : ch * WPAD + c1]

    _evi = [0]

    def evict(dst, src):
        # psum -> sbuf copies alternate between ScalarE and DVE (Pool
        # cannot access PSUM on real HW)
        if _evi[0] % 2 == 0:
            nc.scalar.copy(dst, src)
        else:
            nc.vector.tensor_copy(dst, src)
        _evi[0] += 1

    # sq_full piece-wise fill helper: copy sq rows [h0+r0, h0+r1) of the
    # image into dst_slab rows [r0+roff, r1+roff) of window column wcol.
    def sq_fill(dst_slab, roff, h0, nrows, wcol):
        r0 = max(0, -h0)
        r1 = min(nrows, H - h0)
        while r0 < r1:
            h = h0 + r0
            c, pr = divmod(h, P)
            run = min(r1 - r0, P - pr)
            nc.sync.dma_start(
                out=dst_slab[
                    roff + r0 : roff + r0 + run, wcol + 2 : wcol + 514
                ],
                in_=sq_full[pr : pr + run, c * WPAD + 2 : c * WPAD + 514],
            )
            r0 += run

    with tile.TileContext(nc) as tc:
        with (
            tc.tile_pool(name="io", bufs=2) as io_pool,
            tc.tile_pool(name="fr", bufs=1) as fr_pool,
            tc.tile_pool(name="ps1", bufs=1, space="PSUM") as ps1_pool,
            tc.tile_pool(name="ps2", bufs=1, space="PSUM") as ps2_pool,
            tc.tile_pool(name="ps3", bufs=2, space="PSUM") as ps3_pool,
            tc.tile_pool(name="ps4", bufs=2, space="PSUM") as ps4_pool,
            tc.tile_pool(name="psg", bufs=2, space="PSUM") as psg_pool,
        ):
            for _it in range(loop_k):
                # ---- init: tables, ones rows, margins ----
                nc.sync.dma_start(out=tbl_sb[:], in_=tbl_dram[:])
                # full-slab zero: margins stay zero; edge-window rows that no
                # fill/evict ever writes read as zero (their lhsT coefficients
                # are zeroed too, but CoreSim requires initialized reads)
                for slab in (slab1, slab2, slab3a, slab3b, slab4):
                    nc.gpsimd.memset(slab[:].bitcast(F32), 0.0)
                for slab, orow, nch in (
                    (slab1, ONES1, CPS),
                    (slab2, ONES2, CPS),
                    (slab3a, ONES3, 2 * CPS),
                    (slab4, ONES4, 4 * CPS),
                ):
                    nc.sync.dma_start(
                        out=slab[orow : orow + 1, 0 : nch * WPAD],
                        in_=ones_dram[0 : nch * WPAD],
                    )

                # ---- front-end: sq_full (f32) + sq_bf (bf16) ----
                for c in range(4):
                    A = io_pool.tile([P, W], F32, tag="A")
                    B = io_pool.tile([P, W], F32, tag="B")
                    nc.sync.dma_start(out=A[:], in_=p_dram[c * P : (c + 1) * P, :])
                    if c == 0:
                        nc.sync.dma_start(out=B[1:P, :], in_=p_dram[0 : P - 1, :])
                        nc.sync.dma_start(out=B[0:1, :], in_=p_dram[0:1, :])
                    else:
                        nc.sync.dma_start(
                            out=B[:], in_=p_dram[c * P - 1 : (c + 1) * P - 1, :]
                        )
                    V = fr_pool.tile([P, W], F32, tag="V")
                    K1t = fr_pool.tile([P, W], F32, tag="K1")
                    K2t = fr_pool.tile([P, W], F32, tag="K2")
                    K3t = fr_pool.tile([P, W], F32, tag="K3")
                    K4t = fr_pool.tile([P, W], F32, tag="K4")
                    nc.vector.tensor_tensor(V[:], A[:], B[:], ao.subtract)
                    nc.vector.tensor_scalar(K1t[:], V[:], PI, None, ao.is_ge)
                    nc.vector.tensor_scalar(K2t[:], V[:], 3 * PI, None, ao.is_ge)
                    nc.vector.tensor_scalar(K3t[:], V[:], -PI, None, ao.is_le)
                    nc.vector.tensor_scalar(K4t[:], V[:], -3 * PI, None, ao.is_le)
                    nc.vector.tensor_tensor(K1t[:], K1t[:], K2t[:], ao.add)
                    nc.vector.tensor_tensor(K3t[:], K3t[:], K4t[:], ao.add)
                    nc.vector.tensor_tensor(K1t[:], K1t[:], K3t[:], ao.subtract)
                    nc.vector.scalar_tensor_tensor(
                        V[:], K1t[:], -2 * PI, V[:], ao.mult, ao.add
                    )
                    nc.vector.tensor_tensor(
                        dcol(sq_full, c, 2, 514), V[:], V[:], ao.mult
                    )
                    nc.vector.tensor_tensor(
                        dcol(sq_bf, c, 2, 514), V[:], V[:], ao.mult
                    )
                nc.sync.dma_start(
                    out=osq[:],
                    in_=sq_bf.reshape([P, 4, WPAD])[:, :, 2:514],
                )

                # ---- sections ----
                for s in range(NSEC):
                    # slab1 fills: windows k = 4s+kk, rows [32k-5, +43)
                    for kk in range(CPS):
                        k = CPS * s + kk
                        sq_fill(slab1, 0, 32 * k - 5, 43, kk * WPAD)
                    # slab2 sq fills: rows [32k-4, +40) at partition 80
                    for kk in range(CPS):
                        k = CPS * s + kk
                        sq_fill(slab2, 80, 32 * k - 4, 40, kk * WPAD)

                    # conv1: per chunk
                    for kk in range(CPS):
                        k = CPS * s + kk
                        ps = ps1_pool.tile([P, W], F32, tag="p1")
                        var = _v1(k, NCH - 1, 0)
                        for kw in range(4):
                            dw = kw - 1
                            nc.tensor.matmul(
                                ps[0:M1, :],
                                mm(("c1", kw, 0, var)),
                                slab1[
                                    0:K1, kk * WPAD + 2 + dw : kk * WPAD + 514 + dw
                                ],
                                start=(kw == 0),
                                stop=(kw == 3),
                            )
                        evict(
                            slab2[0:M1, kk * WPAD + 2 : kk * WPAD + 514],
                            ps[0:M1, :],
                        )

                    # c1 out (f32, from slab2)
                    for ch in range(2):
                        nc.scalar.dma_start(
                            out=oc1[s, ch],
                            in_=slab2[40 * ch + 4 : 40 * ch + 36, :]
                            .bitcast(F32)
                            .rearrange("p (k w) -> p k w", k=CPS)[:, :, 2:514],
                        )

                    # slab3b fill via permutation matmul (2 windows/chunk)
                    for kk in range(CPS):
                        ps = psg_pool.tile([P, W], F32, tag="pg")
                        nc.tensor.matmul(
                            ps[0:M3, :],
                            mm(("p3b", 0, 0, "i")),
                            slab2[
                                0:K2, kk * WPAD + 2 : kk * WPAD + 514
                            ],
                            start=True,
                            stop=True,
                        )
                        q0 = 2 * kk
                        evict(
                            slab3b[0:57, q0 * WPAD + 2 : q0 * WPAD + 514],
                            ps[0:57, :],
                        )
                        evict(
                            slab3b[0:57, (q0 + 1) * WPAD + 2 : (q0 + 1) * WPAD + 514],
                            ps[64:121, :],
                        )

                    # conv2: windows q = 8s..8s+8
                    for qq in range(2 * CPS):
                        q = 2 * CPS * s + qq
                        kk = qq // 2
                        par = qq % 2
                        var = _v1(q, 2 * NCH - 1, 0)
                        ps = ps2_pool.tile([P, W], F32, tag="p2")
                        for kw in range(3):
                            dw = 2 * kw - 2
                            nc.tensor.matmul(
                                ps[0:M2, :],
                                mm(("c2", kw, par, var)),
                                slab2[
                                    0:K2, kk * WPAD + 2 + dw : kk * WPAD + 514 + dw
                                ],
                                start=(kw == 0),
                                stop=(kw == 2),
                            )
                        evict(
                            slab3a[0:M2, qq * WPAD + 2 : qq * WPAD + 514],
                            ps[0:M2, :],
                        )

                    # c2 out (f32, from slab3)
                    for ch in range(4):
                        nc.scalar.dma_start(
                            out=oc2[s, ch],
                            in_=slab3a[19 * ch + 1 : 19 * ch + 17, :]
                            .bitcast(F32)
                            .rearrange("p (k w) -> p k w", k=2 * CPS)[:, :, 2:514],
                        )

                    # slab4 c2 via conv2 second pass (4 windows/chunk)
                    for kk in range(CPS):
                        k = CPS * s + kk
                        var = _v1(k, NCH - 1, 0)
                        ps = psg_pool.tile([P, W], F32, tag="pg")
                        for kw in range(3):
                            dw = 2 * kw - 2
                            nc.tensor.matmul(
                                ps[0:M3, :],
                                mm(("c2b", kw, 0, var)),
                                slab2[
                                    0:K2, kk * WPAD + 2 + dw : kk * WPAD + 514 + dw
                                ],
                                start=(kw == 0),
                                stop=(kw == 2),
                            )
                        for w_ in range(4):
                            m0 = 4 * kk + w_
                            evict(
                                slab4[64:96, m0 * WPAD + 2 : m0 * WPAD + 514],
                                ps[32 * w_ : 32 * w_ + 32, :],
                            )
                    # slab4 c1+sq via permutation matmul (4 windows/chunk)
                    for kk in range(CPS):
                        ps = psg_pool.tile([P, W], F32, tag="pg")
                        nc.tensor.matmul(
                            ps[0:M3, :],
                            mm(("p4g", 0, 0, "i")),
                            slab2[
                                0:K2, kk * WPAD + 2 : kk * WPAD + 514
                            ],
                            start=True,
                            stop=True,
                        )
                        for w_ in range(4):
                            m0 = 4 * kk + w_
                            evict(
                                slab4[96:120, m0 * WPAD + 2 : m0 * WPAD + 514],
                                ps[32 * w_ : 32 * w_ + 24, :],
                            )

                    # conv3: pairs q = 8s..8s+8
                    for qq in range(2 * CPS):
                        q = 2 * CPS * s + qq
                        var = _v1(q, 2 * NCH - 1, 0)
                        ps = ps3_pool.tile([P, W], F32, tag="p3")
                        for kw in range(2):
                            dw = 3 * kw - 1
                            nc.tensor.matmul(
                                ps[0:M3, :],
                                mm(("c3a", kw, 0, var)),
                                slab3a[
                                    0:K3A, qq * WPAD + 2 + dw : qq * WPAD + 514 + dw
                                ],
                                start=(kw == 0),
                                stop=False,
                            )
                            nc.tensor.matmul(
                                ps[0:M3, :],
                                mm(("c3b", kw, 0, var)),
                                slab3b[
                                    0:K3B, qq * WPAD + 2 + dw : qq * WPAD + 514 + dw
                                ],
                                start=False,
                                stop=(kw == 1),
                            )
                        m0 = 2 * qq
                        evict(
                            slab4[0:64, m0 * WPAD + 2 : m0 * WPAD + 514],
                            ps[0:64, :],
                        )
                        evict(
                            slab4[0:64, (m0 + 1) * WPAD + 2 : (m0 + 1) * WPAD + 514],
                            ps[64:128, :],
                        )

                    # c3 out (f32, from slab4 c3 block)
                    nc.scalar.dma_start(
                        out=oc3[s],
                        in_=slab4[0:64, :].bitcast(F32)
                        .rearrange("p (k w) -> p k w", k=4 * CPS)[:, :, 2:514],
                    )

                    # conv4: windows m = 16s..16s+16
                    for mm_ in range(4 * CPS):
                        ps = ps4_pool.tile([P, W], F32, tag="p4")
                        nc.tensor.matmul(
                            ps[0:M4, :],
                            mm(("c4", 0, 0, "i")),
                            slab4[
                                0:K4, mm_ * WPAD + 2 : mm_ * WPAD + 514
                            ],
                            start=True,
                            stop=True,
                        )
                        evict(
                            obf4[:, mm_ * WPAD + 2 : mm_ * WPAD + 514], ps[0:M4, :]
                        )

                    # c4 out (bf16)
                    nc.scalar.dma_start(
                        out=oc4[s],
                        in_=obf4.reshape([P, 4 * CPS, WPAD])[:, :, 2:514],
                    )

    nc.compile()
    return nc


_NC_CACHE = None


def _get_nc():
    global _NC_CACHE
    if _NC_CACHE is None:
        _NC_CACHE = build_nc()
    return _NC_CACHE


OUT_NAMES = ["oc1", "oc2", "oc3", "oc4", "osq"]

# final output channel order: c4 x16, then per CH_MAP of the reference concat
_UNIQ31 = (
    [("c4", i) for i in range(16)]
    + [("c3", i) for i in range(8)]
    + [("c2", i) for i in range(4)]
    + [("c1", 0), ("c1", 1), ("sq", 0)]
)
_DUP_TAIL = (
    [("c3", i) for i in range(8)] + [("c2", i) for i in range(4)]
    + [("c1", 0), ("c1", 1), ("sq", 0), ("sq", 0)]
    + [("c1", 0), ("c1", 1), ("sq", 0), ("sq", 0)]
    + [("c2", i) for i in range(4)]
    + [("c1", 0), ("c1", 1), ("sq", 0), ("sq", 0)]
    + [("c1", 0), ("c1", 1), ("sq", 0), ("sq", 0)]
)
CH48 = _UNIQ31[:16] + _DUP_TAIL


def _core_in_maps(inputs):
    tbl = _build_tbl(inputs)
    ones = np.ones(16 * WPAD, np.float32)
    feat = np.asarray(inputs["feature_in"], np.float32)
    return [
        {"p": feat[b, 0], "tbl": tbl, "ones": ones}
        for b in range(feat.shape[0])
    ]


def _assemble48(outs):
    """Device-native outputs (one core) -> [48, H, W] float32."""
    c1 = (
        np.asarray(outs["oc1"], np.float32)
        .transpose(1, 0, 3, 2, 4)
        .reshape(2, H, W)
    )
    c2 = (
        np.asarray(outs["oc2"], np.float32)
        .transpose(1, 0, 3, 2, 4)
        .reshape(4, H, W)
    )
    c3 = (
        np.asarray(outs["oc3"], np.float32)
        .reshape(NSEC, 8, 8, 4 * CPS, W)
        .transpose(1, 0, 3, 2, 4)
        .reshape(8, H, W)
    )
    c4 = (
        np.asarray(outs["oc4"], np.float32)
        .reshape(NSEC, 16, 8, 4 * CPS, W)
        .transpose(1, 0, 3, 2, 4)
        .reshape(16, H, W)
    )
    sq = np.asarray(outs["osq"], np.float32).transpose(1, 0, 2).reshape(1, H, W)
    pl = {"c1": c1, "c2": c2, "c3": c3, "c4": c4, "sq": sq}
    out = np.empty((48, H, W), np.float32)
    for ch, (nm, i) in enumerate(CH48):
        out[ch] = pl[nm][i]
    return out


def _run(inputs, trace=False):
    inputs = {k: np.asarray(v) for k, v in inputs.items()}
    nc = _get_nc()
    in_maps = _core_in_maps(inputs)
    n_cores = len(in_maps)
    res = bass_utils.run_bass_kernel_spmd(
        nc, in_maps, core_ids=list(range(n_cores)), trace=trace
    )
    out = np.stack(
        [_assemble48(res.results[b]) for b in range(n_cores)], axis=0
    )
    return out, res


def kernel(**inputs):
    return _run(inputs, trace=False)[0]
